# revision 7
# baseline (speedup 1.0000x reference)
"""FFM layer (embedding lookup + field-factorization) on 8 trn2 NeuronCores.

Strategy: data-parallel over batch (4096 rows -> 512/core), embedding tables
replicated to every core.  Host packs v and w into one augmented table
(row = 208 v-floats | w | pad to 256 f32 = 1024 B) so one gather per
(tile, field) fetches everything.  HW indirect DMA honors exactly one index
per partition, so each gather is a [128,1]-offset indirect DMA pulling one
1 KiB row per partition; 26 fields x 4 tiles = 104 gathers per core.
VectorE does the strided j-reduction and the quadratic tail:
  out[b] = w0 + sum_f w[idx] + 0.5*(|sum_f e_f|^2 - sum_f |e_f|^2),
with w0 folded into the w column host-side (w + w0/26).
"""

import sys

import numpy as np

FIELD = 26
K = 8
ROW = FIELD * K          # 208 fp32 of v data per table row
RPAD = 256               # padded row length (1024 B, 256 B aligned)
VOCAB = 20000
TOTAL = FIELD * VOCAB    # 520000
B = 4096
NCORES = 8
BC = B // NCORES         # 512 batch rows per core
P = 128
NTILES = BC // P         # 4

_TRN_REPO = "/opt/trn_rl_repo"

_cache = {}


def _build_nc(n_iters=1):
    if _TRN_REPO not in sys.path:
        sys.path.insert(0, _TRN_REPO)
    from concourse import bacc, bass, mybir, tile

    f32 = mybir.dt.float32
    i32 = mybir.dt.int32
    Alu = mybir.AluOpType
    Ax = mybir.AxisListType

    nc = bacc.Bacc("TRN2", target_bir_lowering=False, debug=False)
    idx_d = nc.dram_tensor("idx", [BC, FIELD], i32, kind="ExternalInput")
    vaug_d = nc.dram_tensor("vaug", [TOTAL, RPAD], f32, kind="ExternalInput")
    out_d = nc.dram_tensor("out", [BC, 1], f32, kind="ExternalOutput")

    with tile.TileContext(nc) as tc:
        with tc.tile_pool(name="const", bufs=2) as cpool, \
             tc.tile_pool(name="vgp", bufs=2) as vpool:
            for _ in range(n_iters):
                # partition p holds the 4 tiles' rows (t*128+p) of idx
                idx_sb = cpool.tile([P, NTILES, FIELD], i32, tag="idx")
                nc.sync.dma_start(
                    out=idx_sb[:],
                    in_=idx_d[:, :].rearrange("(t p) f -> p t f", p=P),
                )

                wsum_all = cpool.tile([P, NTILES], f32, tag="ws")
                # e_all[p, t, f, k] = sum_j v[idx[t*128+p, f], j, k]
                e_all = cpool.tile([P, NTILES, FIELD, K], f32, tag="e")

                for t in range(NTILES):
                    vg = vpool.tile([P, FIELD, RPAD], f32, tag="vg")
                    for f in range(FIELD):
                        nc.gpsimd.indirect_dma_start(
                            out=vg[:, f, :],
                            out_offset=None,
                            in_=vaug_d[:, :],
                            in_offset=bass.IndirectOffsetOnAxis(
                                ap=idx_sb[:, t, f:f + 1], axis=0
                            ),
                        )
                    nc.vector.tensor_reduce(
                        out=e_all[:, t],
                        in_=vg[:, :, :ROW].rearrange(
                            "p f (j k) -> p f k j", j=FIELD, k=K
                        ),
                        axis=Ax.X,
                        op=Alu.add,
                    )
                    # first-order sums (incl. folded w0) from the w column
                    nc.vector.tensor_reduce(
                        out=wsum_all[:, t:t + 1],
                        in_=vg[:, :, ROW:ROW + 1].rearrange(
                            "p f one -> p one f"
                        ),
                        axis=Ax.X,
                        op=Alu.add,
                    )

                # batched tail over all 4 tiles
                esq = cpool.tile([P, NTILES, FIELD * K], f32, tag="esq")
                nc.vector.tensor_tensor(
                    out=esq[:],
                    in0=e_all[:].rearrange("p t f k -> p t (f k)"),
                    in1=e_all[:].rearrange("p t f k -> p t (f k)"),
                    op=Alu.mult,
                )
                sqs = cpool.tile([P, NTILES], f32, tag="sqs")
                nc.vector.tensor_reduce(
                    out=sqs[:], in_=esq[:], axis=Ax.X, op=Alu.add
                )
                s_all = cpool.tile([P, NTILES, K], f32, tag="s")
                nc.vector.tensor_reduce(
                    out=s_all[:],
                    in_=e_all[:].rearrange("p t f k -> p t k f"),
                    axis=Ax.X,
                    op=Alu.add,
                )
                ssq = cpool.tile([P, NTILES, K], f32, tag="ssq")
                nc.vector.tensor_tensor(
                    out=ssq[:], in0=s_all[:], in1=s_all[:], op=Alu.mult
                )
                s2s = cpool.tile([P, NTILES], f32, tag="s2s")
                nc.vector.tensor_reduce(
                    out=s2s[:], in_=ssq[:], axis=Ax.X, op=Alu.add
                )
                d0 = cpool.tile([P, NTILES], f32, tag="d0")
                nc.vector.tensor_tensor(
                    out=d0[:], in0=s2s[:], in1=sqs[:], op=Alu.subtract
                )
                d0h = cpool.tile([P, NTILES], f32, tag="d0h")
                nc.vector.tensor_scalar_mul(d0h[:], d0[:], 0.5)
                out_all = cpool.tile([P, NTILES], f32, tag="oa")
                nc.vector.tensor_tensor(
                    out=out_all[:], in0=d0h[:], in1=wsum_all[:], op=Alu.add
                )
                # single store: out[t*128+p] = out_all[p, t]
                nc.sync.dma_start(
                    out=out_d[:, :].rearrange("(t p) one -> p (t one)", p=P),
                    in_=out_all[:],
                )
    nc.compile()
    return nc


def get_nc():
    if "nc" not in _cache:
        _cache["nc"] = _build_nc()
    return _cache["nc"]


def make_in_maps(inputs, offsets, w0, w, v):
    idx = (np.asarray(inputs, dtype=np.int64)
           + np.asarray(offsets, dtype=np.int64)[None, :]).astype(np.int32)
    # augmented, 1 KiB-aligned table row: [v row (208) | w + w0/26 | zeros]
    vaug = np.zeros((TOTAL, RPAD), dtype=np.float32)
    vaug[:, :ROW] = np.asarray(v, dtype=np.float32).reshape(TOTAL, ROW)
    vaug[:, ROW] = (np.asarray(w, dtype=np.float32).reshape(TOTAL)
                    + np.float32(np.asarray(w0, np.float32).reshape(()) / FIELD))
    return [
        {"idx": np.ascontiguousarray(idx[i * BC:(i + 1) * BC]), "vaug": vaug}
        for i in range(NCORES)
    ]


def kernel(inputs, offsets, w0, w, v):
    if _TRN_REPO not in sys.path:
        sys.path.insert(0, _TRN_REPO)
    from concourse.bass_utils import run_bass_kernel_spmd

    nc = get_nc()
    in_maps = make_in_maps(inputs, offsets, w0, w, v)
    res = run_bass_kernel_spmd(nc, in_maps, list(range(NCORES)))
    out = np.concatenate(
        [np.asarray(res.results[i]["out"]) for i in range(NCORES)], axis=0
    )
    return out.astype(np.float32)


# revision 12
# speedup vs baseline: 4.0376x; 4.0376x over previous
"""FFM layer (embedding lookup + field-factorization) on 8 trn2 NeuronCores.

Strategy: data-parallel over batch (4096 rows -> 512/core), embedding tables
replicated to every core.  Host packs v and w into one augmented table
(row = 208 v-floats | w | pad to 256 f32 = 1024 B).  Lookups use the SWDGE
dma_gather custom instruction, one per field: indices are field-local
(< 20000, int16) into the field's subtable slice, 512 indices per gather
(the core's whole batch shard), so the ~1 us per-DMA fixed cost is paid 26
times instead of once per 128 rows.  Index ordinal i = batch row lands at
dest [i % 128, i // 128, :], which is exactly the (partition, batch-tile)
layout the compute wants.  VectorE then does the strided j-reduction and
the quadratic tail:
  out[b] = w0 + sum_f w[idx] + 0.5*(|sum_f e_f|^2 - sum_f |e_f|^2),
with w0 folded into the packed w column host-side (w + w0/26).
Fields are processed in 4 groups so gathers of group g+1 overlap VectorE
reduction of group g.
"""

import sys

import numpy as np

FIELD = 26
K = 8
ROW = FIELD * K          # 208 fp32 of v data per table row
RPAD = 256               # padded row length (1024 B, 256 B aligned)
VOCAB = 20000
TOTAL = FIELD * VOCAB    # 520000
B = 4096
NCORES = 8
BC = B // NCORES         # 512 batch rows per core
P = 128
NTILES = BC // P         # 4
NSLOT = BC // 16         # 32 int16 index slots per idx partition

# field groups for gather/compute pipelining
GROUPS = [list(range(s, min(s + 7, FIELD))) for s in range(0, FIELD, 7)]

_TRN_REPO = "/opt/trn_rl_repo"

_cache = {}


def _build_nc(n_iters=1):
    if _TRN_REPO not in sys.path:
        sys.path.insert(0, _TRN_REPO)
    from concourse import bacc, mybir, tile

    f32 = mybir.dt.float32
    i16 = mybir.dt.int16
    Alu = mybir.AluOpType
    Ax = mybir.AxisListType

    nc = bacc.Bacc("TRN2", target_bir_lowering=False, debug=False)
    # idx16[p, f, s] = int16 field-local index of batch row s*16+(p%16),
    # field f -- 16-partition wrap replicated to 128 host-side
    idx_d = nc.dram_tensor("idx16", [P, FIELD, NSLOT], i16,
                           kind="ExternalInput")
    vaug_d = nc.dram_tensor("vaug", [TOTAL, RPAD], f32, kind="ExternalInput")
    out_d = nc.dram_tensor("out", [BC, 1], f32, kind="ExternalOutput")

    NG = len(GROUPS)

    with tile.TileContext(nc) as tc:
        with tc.tile_pool(name="const", bufs=2) as cpool, \
             tc.tile_pool(name="vgp", bufs=2) as vpool:
            for _ in range(n_iters):
                idx_sb = cpool.tile([P, FIELD, NSLOT], i16, tag="idx")
                nc.sync.dma_start(out=idx_sb[:], in_=idx_d[:, :, :])

                # e_all[p, t, f, k] = sum_j v[idx[t*128+p, f], j, k]
                e_all = cpool.tile([P, NTILES, FIELD, K], f32, tag="e")
                wpart = cpool.tile([P, NG, NTILES], f32, tag="wp")

                for gi, grp in enumerate(GROUPS):
                    gsz = len(grp)
                    vg = vpool.tile([P, gsz, NTILES, RPAD], f32,
                                    tag=f"vg{gi % 2}")
                    for j, f in enumerate(grp):
                        nc.gpsimd.dma_gather(
                            out_ap=vg[:, j],
                            in_ap=vaug_d[f * VOCAB:(f + 1) * VOCAB, :],
                            idxs_ap=idx_sb[:, f, :],
                            num_idxs=BC,
                            num_idxs_reg=BC,
                            elem_size=RPAD,
                        )
                    # j-reduction for this group's fields, all 4 tiles
                    nc.vector.tensor_reduce(
                        out=e_all[:, :, grp[0]:grp[0] + gsz, :]
                        .rearrange("p t f k -> p f t k"),
                        in_=vg[:, :, :, :ROW].rearrange(
                            "p f t (j k) -> p f t k j", j=FIELD, k=K
                        ),
                        axis=Ax.X,
                        op=Alu.add,
                    )
                    # first-order partials from the packed w column
                    nc.vector.tensor_reduce(
                        out=wpart[:, gi],
                        in_=vg[:, :, :, ROW].rearrange("p f t -> p t f"),
                        axis=Ax.X,
                        op=Alu.add,
                    )

                wsum_all = cpool.tile([P, NTILES], f32, tag="ws")
                nc.vector.tensor_reduce(
                    out=wsum_all[:],
                    in_=wpart[:].rearrange("p g t -> p t g"),
                    axis=Ax.X,
                    op=Alu.add,
                )

                # batched tail over all 4 tiles
                esq = cpool.tile([P, NTILES, FIELD * K], f32, tag="esq")
                nc.vector.tensor_tensor(
                    out=esq[:],
                    in0=e_all[:].rearrange("p t f k -> p t (f k)"),
                    in1=e_all[:].rearrange("p t f k -> p t (f k)"),
                    op=Alu.mult,
                )
                sqs = cpool.tile([P, NTILES], f32, tag="sqs")
                nc.vector.tensor_reduce(
                    out=sqs[:], in_=esq[:], axis=Ax.X, op=Alu.add
                )
                s_all = cpool.tile([P, NTILES, K], f32, tag="s")
                nc.vector.tensor_reduce(
                    out=s_all[:],
                    in_=e_all[:].rearrange("p t f k -> p t k f"),
                    axis=Ax.X,
                    op=Alu.add,
                )
                ssq = cpool.tile([P, NTILES, K], f32, tag="ssq")
                nc.vector.tensor_tensor(
                    out=ssq[:], in0=s_all[:], in1=s_all[:], op=Alu.mult
                )
                s2s = cpool.tile([P, NTILES], f32, tag="s2s")
                nc.vector.tensor_reduce(
                    out=s2s[:], in_=ssq[:], axis=Ax.X, op=Alu.add
                )
                d0 = cpool.tile([P, NTILES], f32, tag="d0")
                nc.vector.tensor_tensor(
                    out=d0[:], in0=s2s[:], in1=sqs[:], op=Alu.subtract
                )
                d0h = cpool.tile([P, NTILES], f32, tag="d0h")
                nc.vector.tensor_scalar_mul(d0h[:], d0[:], 0.5)
                out_all = cpool.tile([P, NTILES], f32, tag="oa")
                nc.vector.tensor_tensor(
                    out=out_all[:], in0=d0h[:], in1=wsum_all[:], op=Alu.add
                )
                # single store: out[t*128+p] = out_all[p, t]
                nc.sync.dma_start(
                    out=out_d[:, :].rearrange("(t p) one -> p (t one)", p=P),
                    in_=out_all[:],
                )
    nc.compile()
    return nc


def get_nc():
    if "nc" not in _cache:
        _cache["nc"] = _build_nc()
    return _cache["nc"]


def make_in_maps(inputs, offsets, w0, w, v):
    del offsets  # folded into the per-field subtable slicing
    inp = np.asarray(inputs)
    # field-local int16 indices, wrapped: idx16[f, p, s] = inputs[s*16+p, f]
    idx16 = np.ascontiguousarray(
        inp.astype(np.int16).reshape(NCORES, BC, FIELD)
    )
    # augmented, 1 KiB-aligned table row: [v row (208) | w + w0/26 | zeros]
    vaug = np.zeros((TOTAL, RPAD), dtype=np.float32)
    vaug[:, :ROW] = np.asarray(v, dtype=np.float32).reshape(TOTAL, ROW)
    vaug[:, ROW] = (np.asarray(w, dtype=np.float32).reshape(TOTAL)
                    + np.float32(np.asarray(w0, np.float32).reshape(()) / FIELD))
    maps = []
    for i in range(NCORES):
        shard = idx16[i]                       # [BC, FIELD]
        wrapped = shard.reshape(NSLOT, 16, FIELD).transpose(1, 2, 0)
        # [16, FIELD, NSLOT] -> replicate to 128 partitions
        rep = np.ascontiguousarray(np.tile(wrapped, (NCORES, 1, 1)))
        maps.append({"idx16": rep, "vaug": vaug})
    return maps


def kernel(inputs, offsets, w0, w, v):
    if _TRN_REPO not in sys.path:
        sys.path.insert(0, _TRN_REPO)
    from concourse.bass_utils import run_bass_kernel_spmd

    nc = get_nc()
    in_maps = make_in_maps(inputs, offsets, w0, w, v)
    res = run_bass_kernel_spmd(nc, in_maps, list(range(NCORES)))
    out = np.concatenate(
        [np.asarray(res.results[i]["out"]) for i in range(NCORES)], axis=0
    )
    return out.astype(np.float32)
